# revision 102
# baseline (speedup 1.0000x reference)
"""Trainium2 Bass kernel for nn_MultiHeadAttention_87110526697836 (v3).

Data-parallel over batch B=8 across 8 NeuronCores. Per core: causal MHA with
relative-position biases (max_dist=16), fused softmax, output projection,
residual, LayerNorm.

v3 changes over v2:
  - fp8(e4m3) DoubleRow matmuls for the Q/K/V projections and the score
    matmuls (0.5 cycles/row on PE).  Embeddings repacked to [32, 16, T]
    (feature split 512 -> 16 groups of 32 partitions) through a DRAM round
    trip so the DoubleRow pair dim lives in the free axis.
  - the pe_k[0] "far band" fold is dropped entirely: exp(qh.pe_k0) is a
    per-row constant factor that cancels in the softmax (and in rel-v after
    normalization), so only the 16-wide near-band correction remains.
  - the 1/sqrt(DH) scale is folded into the exp (activation scale=0.125)
    instead of the weights, keeping fp8 operand ranges healthy.
  - AV split into far-field (reads the uncorrected exp scores immediately)
    and band part (reads the corrected 144-wide windows re-read into a
    separate bw tile), so AV no longer serializes on the band DMA chain.
  - exp split between ACT (true Exp) and DVE (Schraudolph int16-bitcast
    approx, one tensor_scalar per tile) to break the ACT throughput wall.
  - softmax denominator broadcast and the wband regroup ride the scalar /
    vector HWDGE queues; only the scatter-accumulate stays on SWDGE.
"""

import math

import numpy as np
import ml_dtypes

import bass_rust
import concourse.bass as bass
import concourse.mybir as mybir
import concourse.tile as tile
from concourse.bass_utils import run_bass_kernel_spmd

F32 = mybir.dt.float32
BF16 = mybir.dt.bfloat16
F8 = mybir.dt.float8e4
I16 = mybir.dt.int16
DR = mybir.MatmulPerfMode.DoubleRow

H = 8
M = 16
U = 512
DH = U // H        # 64
T = 1024
B = 8
EPS = 1e-3
NT = T // 128      # 8 t-chunks / s-chunks
WTW = 1040         # per-chunk padded width of the w tile (128*8 + 16)
CSTRIDE = WTW + 128  # step between (c, s0_c) anchors inside a wt tile
EXT_W = 144        # staged window width per chunk
RW = NT * EXT_W    # staged window row width (1152)
HSTG = 128 * RW    # per-head staging size (147456)

SM_SCALE = 1.0 / 8.0          # 1/sqrt(DH), folded into the exp
EXP_A = SM_SCALE * 128.0 / math.log(2.0)   # Schraudolph scale for bf16 bits
EXP_B = 16250.5                            # 16256 - 5.5 (centered)
# exp pieces (c,th) whose exp runs on DVE via the Schraudolph int16 trick:
# every 3rd piece, interleaved so ACT and DVE overlap
def _dve_piece(idx, h=2):
    if h < 2:
        return idx % 2 == 1
    return idx in (1, 3, 5, 8, 10)

# ---------------------------------------------------------------------------
# Tile-framework workarounds: walrus accepts a limited number of sync-wait
# commands per instruction; split excess waits onto same-engine NOPs.
# ---------------------------------------------------------------------------


def _patched_drain_and_barrier(self, tick_clock, wait_clock):
    from bass_rust import ScopedClock

    nc = self.nc
    nop_inst = nc.sync.nop()
    wait_clock.add_sem_waits(nop_inst.ins, ScopedClock({None: tick_clock.global_clock}))
    si = nop_inst.ins.sync_info
    if si is not None and si.on_wait and len(si.on_wait) > 1:
        waits = list(si.on_wait)
        nop_inst.ins.sync_info = bass_rust.SyncInfo(
            on_wait=[waits[0]], on_update=list(si.on_update or [])
        )
        for w in waits[1:]:
            extra = nc.sync.nop()
            extra.ins.sync_info = bass_rust.SyncInfo(on_wait=[w], on_update=[])
    nc.sync.drain()
    nc.all_engine_barrier()
    popped = nc._tile_sem_poison_stack.pop()
    assert popped is self._sem_poison
    nc.clear_and_free_semaphores(list(self.sems.allocated().values()))
    nc.all_engine_barrier()


tile.TileContext._drain_and_barrier = _patched_drain_and_barrier


def split_excess_waits(nc, limit=1):
    counter = 0
    for f in nc.m.functions:
        for bb in f.blocks:
            insts = bb.instructions
            out = []
            changed = False
            for ins in insts:
                si = ins.sync_info
                if si is not None and si.on_wait is not None and len(si.on_wait) > limit:
                    waits = list(si.on_wait)
                    extra, keep = waits[:-limit], waits[-limit:]
                    for w in extra:
                        counter += 1
                        nop = mybir.InstNoOp(name=f"waitsplit-{counter}", ins=[], outs=[])
                        nop.engine = ins.engine
                        nop.sync_info = bass_rust.SyncInfo(on_wait=[w], on_update=[])
                        nc.inst_map[nop.name] = nop
                        out.append(nop)
                    ins.sync_info = bass_rust.SyncInfo(
                        on_wait=keep, on_update=list(si.on_update or [])
                    )
                    changed = True
                out.append(ins)
            if changed:
                bb.instructions = out
    return counter


def dram_ap(t, offset, dims):
    return bass.AP(tensor=t, offset=offset, ap=[list(d) for d in dims])


# ---------------------------------------------------------------------------
# Kernel builder
# ---------------------------------------------------------------------------


def build_nc(apply_gamma_beta: bool, has_bv: bool):
    nc = bass.Bass(num_swdge_queues=4)

    p_qT = nc.declare_dram_parameter("qT8", [128, 4096], F8, isOutput=False)
    p_kT = nc.declare_dram_parameter("kT8", [128, 4096], F8, isOutput=False)
    p_vT = nc.declare_dram_parameter("vT8", [128, 4096], F8, isOutput=False)
    p_qn = nc.declare_dram_parameter("qn", [T, U], BF16, isOutput=False)
    p_Wq = nc.declare_dram_parameter("Wq8", [128, 2048], F8, isOutput=False)
    p_Wk = nc.declare_dram_parameter("Wk8", [128, 2048], F8, isOutput=False)
    p_Wv = nc.declare_dram_parameter("Wv8", [128, 2048], F8, isOutput=False)
    p_Wo = nc.declare_dram_parameter("Wo", [U, U], BF16, isOutput=False)
    p_bq = nc.declare_dram_parameter("bq_c", [U, 1], F32, isOutput=False)
    p_bk = nc.declare_dram_parameter("bk_c", [U, 1], F32, isOutput=False)
    p_bv = nc.declare_dram_parameter("bv_row", [1, U], BF16, isOutput=False)
    p_bo = nc.declare_dram_parameter("bo_row", [1, U], BF16, isOutput=False)
    p_dpk = nc.declare_dram_parameter("dpk8", [32, 32], F8, isOutput=False)
    p_dpvt = nc.declare_dram_parameter("dpvr", [M, DH], BF16, isOutput=False)
    p_bigm = nc.declare_dram_parameter("bigm", [128, 384], BF16, isOutput=False)
    p_idf = nc.declare_dram_parameter("identf", [128, 128], F32, isOutput=False)
    p_cm = nc.declare_dram_parameter("cmask", [128, 128], BF16, isOutput=False)
    p_gam = nc.declare_dram_parameter("gamma_r", [1, U], F32, isOutput=False)
    p_bet = nc.declare_dram_parameter("beta_r", [1, U], F32, isOutput=False)
    p_out = nc.declare_dram_parameter("out", [T, U], BF16, isOutput=True)

    win_stage = [nc.dram_tensor(f"win_stage{h}", [HSTG], BF16) for h in range(H)]
    rden_stage = [nc.dram_tensor(f"rden_stage{h}", [T], F32) for h in range(H)]
    wb_stage = [nc.dram_tensor(f"wb_stage{h}", [16384], BF16) for h in range(H)]
    q_stage = nc.dram_tensor("q_stage", [U * T], F8)
    k_stage = nc.dram_tensor("k_stage", [U * T], F8)

    with tile.TileContext(nc) as tc:
        import contextlib

        cstack = contextlib.ExitStack()
        consts = cstack.enter_context(tc.tile_pool(name="consts", bufs=1))
        emb_pool = cstack.enter_context(tc.tile_pool(name="emb", bufs=1))
        band_pool = cstack.enter_context(tc.tile_pool(name="band", bufs=4))
        mrg_pool = cstack.enter_context(tc.tile_pool(name="mrg", bufs=1))
        nrm_pool = cstack.enter_context(tc.tile_pool(name="nrm", bufs=3))
        ln_pool = cstack.enter_context(tc.tile_pool(name="ln", bufs=4))
        y_pool = cstack.enter_context(tc.tile_pool(name="ypool", bufs=1))
        wt_pool = cstack.enter_context(tc.tile_pool(name="wt", bufs=3))
        bw_pool = cstack.enter_context(tc.tile_pool(name="bw", bufs=3))

        ps_main = cstack.enter_context(tc.tile_pool(name="ps_main", bufs=3, space="PSUM"))
        ps_o = cstack.enter_context(tc.tile_pool(name="ps_o", bufs=2, space="PSUM"))
        ps_band = cstack.enter_context(tc.tile_pool(name="ps_band", bufs=1, space="PSUM"))

        # ---- constant loads ----------------------------------------------
        def load_f8(param, name, width, q=None):
            t_ = consts.tile([128, 2, 2, width], F8, tag=f"f8{name}", name=f"f8{name}")
            (q or nc.sync).dma_start(
                out=t_, in_=dram_ap(param, 0, [[4 * width, 128], [1, 4 * width]])
            )
            return t_

        # tiny constants first so early consumers aren't queued behind MBs
        bq_sb = consts.tile([128, 4], F32, tag="bq")
        nc.sync.dma_start(out=bq_sb, in_=dram_ap(p_bq, 0, [[1, 128], [128, 4]]))
        bk_sb = consts.tile([128, 4], F32, tag="bk")
        nc.sync.dma_start(out=bk_sb, in_=dram_ap(p_bk, 0, [[1, 128], [128, 4]]))
        # big operands split across the SP and ACT HWDGE queues
        Wq_sb = load_f8(p_Wq, "wq", 512)
        qT_sb = load_f8(p_qT, "xq", 1024, q=nc.gpsimd)
        Wk_sb = load_f8(p_Wk, "wk", 512)
        kT_sb = load_f8(p_kT, "xk", 1024, q=nc.gpsimd)
        Wv_sb = load_f8(p_Wv, "wv", 512)
        cm_sb = consts.tile([128, 128], BF16, tag="cmask")
        nc.gpsimd.dma_start(out=cm_sb, in_=p_cm[:, :])
        dpk_sb = consts.tile([32, 2, 16], F8, tag="dpk8")
        nc.gpsimd.dma_start(out=dpk_sb, in_=dram_ap(p_dpk, 0, [[32, 32], [1, 32]]))
        vT_sb = load_f8(p_vT, "xv", 1024, q=nc.gpsimd)
        bigm_sb = consts.tile([128, 384], BF16, tag="bigm")
        nc.sync.dma_start(out=bigm_sb, in_=p_bigm[:, :])
        idf_sb = consts.tile([128, 128], F32, tag="idf")
        nc.sync.dma_start(out=idf_sb, in_=p_idf[:, :])
        dpvt_sb = consts.tile([M, DH], BF16, tag="dpvt")
        nc.sync.dma_start(out=dpvt_sb, in_=p_dpvt[:, :])

        Wo_sb = consts.tile([128, 4, U], BF16, tag="wo")
        nc.gpsimd.dma_start(
            out=Wo_sb, in_=dram_ap(p_Wo, 0, [[U, 128], [128 * U, 4], [1, U]])
        )

        if has_bv:
            bv_sb = consts.tile([1, U], BF16, tag="bv")
            nc.sync.dma_start(out=bv_sb, in_=p_bv[:, :])
        bo_sb = consts.tile([1, U], BF16, tag="bo")
        nc.sync.dma_start(out=bo_sb, in_=p_bo[:, :])

        if apply_gamma_beta:
            gam_bc = consts.tile([128, U], F32, tag="gambc")
            nc.sync.dma_start(out=gam_bc, in_=dram_ap(p_gam, 0, [[0, 128], [1, U]]))
            bet_bc = consts.tile([128, U], F32, tag="betbc")
            nc.sync.dma_start(out=bet_bc, in_=dram_ap(p_bet, 0, [[0, 128], [1, U]]))

        ones_row = consts.tile([1, 128], BF16, tag="ones")
        nc.vector.memset(ones_row, 1.0)
        eps_sb = consts.tile([128, 1], F32, tag="eps")
        nc.vector.memset(eps_sb, EPS)

        # ---- phase 1: projections (fp8 DoubleRow) ------------------------
        # q/k: psum [128 f(co-strip), 512 t] -> relu -> q_embT8 [128,4co,T]
        # then repacked through DRAM into [32, 16 fhi, T] (f = 32*fhi + p).
        q_embT8 = emb_pool.tile([128, 4, T], F8, tag="qeT", name="qeT")
        k_embT8 = emb_pool.tile([128, 4, T], F8, tag="keT", name="keT")
        q_emb8 = emb_pool.tile([32, 16, T], F8, tag="qe8", name="qe8")
        k_emb8 = emb_pool.tile([32, 16, T], F8, tag="ke8", name="ke8")
        v_pad = emb_pool.tile([128, NT, H, 66], BF16, tag="vp")

        def emit_qk_co(W_sb, xT_sb, b_sb, embT8, emb8, stage, co, relu_dve):
            for th in range(2):
                ps = ps_main.tile([128, 512], F32, tag="msc")
                for j in range(2):
                    nc.tensor.matmul(
                        ps,
                        lhsT=W_sb[:, j, :, co * 128:(co + 1) * 128],
                        rhs=xT_sb[:, j, :, th * 512:(th + 1) * 512],
                        start=(j == 0),
                        stop=(j == 1),
                        perf_mode=DR,
                    )
                if relu_dve:
                    nc.vector.tensor_scalar(
                        out=embT8[:, co, th * 512:(th + 1) * 512],
                        in0=ps,
                        scalar1=b_sb[:, co:co + 1],
                        scalar2=0.0,
                        op0=mybir.AluOpType.add,
                        op1=mybir.AluOpType.max,
                    )
                else:
                    nc.scalar.activation(
                        out=embT8[:, co, th * 512:(th + 1) * 512],
                        in_=ps,
                        func=mybir.ActivationFunctionType.Relu,
                        bias=b_sb[:, co:co + 1],
                    )
            # stage this co strip out as soon as it's done
            nc.sync.dma_start(
                out=dram_ap(stage, co * 128 * T, [[T, 128], [1, T]]),
                in_=embT8[:, co, :],
            )
            if co == 3:
                nc.sync.dma_start(
                    out=emb8[:, 4:16, :],
                    in_=dram_ap(stage, 4 * 32 * T,
                                [[T, 32], [32 * T, 12], [1, T]]),
                )
            if co == 0:
                nc.sync.dma_start(
                    out=emb8[:, 0:4, :],
                    in_=dram_ap(stage, 0, [[T, 32], [32 * T, 4], [1, T]]),
                )

        def emit_v_ti(ti):
            ps = ps_main.tile([128, 512], F32, tag="msc")
            for j in range(2):
                nc.tensor.matmul(
                    ps,
                    lhsT=vT_sb[:, j, :, ti * 128:(ti + 1) * 128],
                    rhs=Wv_sb[:, j, :, :],
                    start=(j == 0),
                    stop=(j == 1) if not has_bv else False,
                    perf_mode=DR,
                )
            if has_bv:
                nc.tensor.matmul(ps, lhsT=ones_row, rhs=bv_sb, start=False, stop=True)
            if ti % 2 == 0:
                nc.scalar.activation(
                    out=v_pad[:, ti, :, 0:64],
                    in_=ps.rearrange("p (a b) -> p a b", a=H),
                    func=mybir.ActivationFunctionType.Relu,
                )
            else:
                nc.vector.tensor_scalar_max(
                    out=v_pad[:, ti, :, 0:64],
                    in0=ps.rearrange("p (a b) -> p a b", a=H),
                    scalar1=0.0,
                )

        mergedT = []
        for ci in range(4):
            mg_ci = mrg_pool.tile([128, T], BF16, tag=f"mg{ci}", name=f"mg{ci}")
            mergedT.append(mg_ci)
        qn_sb = consts.tile([128, NT, U], BF16, tag="qn")

        # ---- phase 2: attention per head, software-pipelined -------------
        state = {}

        SCORE_PIECES = []
        for c in range(NT):
            s0 = c * 128
            for th in range(2):
                lo = max(th * 512, s0)
                hi = (th + 1) * 512
                if lo < hi:
                    SCORE_PIECES.append((c, th, lo, hi))

        def emit_scores_part(h, a, b):
            """Stage A (part): score matmuls (fp8 DR) + exp (ACT/DVE)."""
            fhi0 = 2 * h
            wt_t = state[h][0]
            hb = 64 * (h % 2)
            for idx in range(a, b):
                c, th, lo, hi = SCORE_PIECES[idx]
                s0 = c * 128
                ps = ps_main.tile([128, 512], F32, tag="msc")
                if h < 2:
                    # first heads: straight from the un-repacked co-0 strip
                    # (fp8 non-DR, 1 cycle/row) -- skips the repack round trip
                    nc.tensor.matmul(
                        ps[:, lo - th * 512:512],
                        lhsT=k_embT8[hb:hb + 64, 0, s0:s0 + 128],
                        rhs=q_embT8[hb:hb + 64, 0, lo:hi],
                        start=True,
                        stop=True,
                        skip_group_check=True,
                    )
                else:
                    nc.tensor.matmul(
                        ps[:, lo - th * 512:512],
                        lhsT=k_emb8[:, fhi0:fhi0 + 2, s0:s0 + 128],
                        rhs=q_emb8[:, fhi0:fhi0 + 2, lo:hi],
                        start=True,
                        stop=True,
                        perf_mode=DR,
                        skip_group_check=True,
                    )
                if _dve_piece(idx, h):
                    # Schraudolph: bf16 bits of exp(x/8) ~= int16(a*x + b)
                    nc.vector.tensor_scalar(
                        out=wt_t[:, c, lo:hi].bitcast(I16),
                        in0=ps[:, lo - th * 512:512],
                        scalar1=float(EXP_A),
                        scalar2=float(EXP_B),
                        op0=mybir.AluOpType.mult,
                        op1=mybir.AluOpType.add,
                    )
                else:
                    nc.scalar.activation(
                        out=wt_t[:, c, lo:hi],
                        in_=ps[:, lo - th * 512:512],
                        func=mybir.ActivationFunctionType.Exp,
                        scale=float(SM_SCALE),
                    )

        def emit_scores_open(h):
            wt_t = wt_pool.tile([128, NT, WTW], BF16, tag="wt")
            state[h] = [wt_t, None, None, None, None]

        def emit_scores_close(h):
            """mask + window staging write + band-runs read."""
            wt_t = state[h][0]
            # pad columns of the last chunk are never computed; zero them
            nc.scalar.memzero(wt_t[:, NT - 1, T:WTW])
            # causal mask for all 8 diagonal blocks: multiply by I(x >= p)
            diag_view = bass.AP(
                tensor=wt_t.tensor,
                offset=wt_t.offset,
                ap=[list(wt_t.ap[0]), [CSTRIDE, NT], [1, 128]],
            )
            cm_bcast = bass.AP(
                tensor=cm_sb.tensor,
                offset=cm_sb.offset,
                ap=[list(cm_sb.ap[0]), [0, NT], [1, 128]],
            )
            nc.vector.tensor_tensor(
                out=diag_view, in0=diag_view, in1=cm_bcast,
                op=mybir.AluOpType.mult,
            )
            # stage the 144-wide diagonal windows of all 8 chunks: one DMA
            wstg = win_stage[h]
            win_view = bass.AP(
                tensor=wt_t.tensor,
                offset=wt_t.offset,
                ap=[list(wt_t.ap[0]), [CSTRIDE, NT], [1, EXT_W]],
            )
            nc.scalar.dma_start(
                out=dram_ap(wstg, 0, [[RW, 128], [1, RW]]),
                in_=win_view,
            )
            # band runs: B9[p, 1+c, u] = w[s=128c+p, t=s+u]
            B9 = band_pool.tile([128, 9, 16], BF16, tag="b9")
            nc.scalar.memzero(B9[:, 0, :])
            nc.sync.dma_start(
                out=B9[:, 1:9, :],
                in_=dram_ap(wstg, 0, [[RW + 1, 128], [EXT_W, NT], [1, 16]]),
            )
            state[h][1] = B9

        # far-field AV ranges: per th, list of (c, lo, hi) in global t coords
        FAR = {0: [], 1: []}
        BANDAV = {0: [], 1: []}
        for c in range(NT):
            s0 = c * 128
            flo = s0 + EXT_W
            for th in range(2):
                wlo, whi = th * 512, (th + 1) * 512
                lo, hi = max(flo, wlo), min(1024, whi)
                if lo < hi:
                    FAR[th].append((c, lo, hi))
                blo, bhi = max(s0, wlo), min(s0 + EXT_W, 1024, whi)
                if blo < bhi:
                    BANDAV[th].append((c, blo, bhi))

        def emit_far_av(h):
            """Far-field AV + den: does not depend on the band correction."""
            st = state[h]
            wt_t = st[0]
            po = ps_o.tile([65, 2, 512], F32, tag="po")
            pof = bass.AP(
                tensor=po.tensor, offset=po.offset,
                ap=[list(po.ap[0]), [1, 1024]],
            )
            st[2] = pof
            for th in range(2):
                for i, (c, lo, hi) in enumerate(FAR[th]):
                    nc.tensor.matmul(
                        pof[:, lo:hi],
                        lhsT=v_pad[:, c, h, 0:65],
                        rhs=wt_t[:, c, lo:hi],
                        start=(i == 0),
                        stop=False,
                        skip_group_check=True,
                    )

        def emit_band(h):
            """Stage B: rel-k band correction, scatter-accumulate +
            corrected window re-read, wband staging."""
            st = state[h]
            wt_t, B9 = st[0], st[1]
            fhi0 = 2 * h
            wstg = win_stage[h]
            # G in t-anchored layout: G2[t, c, u] = exp(qh[t] . dpk_rev[u])
            psb = ps_band.tile([128, 24, 16], F32, tag="bandps")
            for c in range(NT):
                nc.tensor.matmul(
                    psb[:, 8 + c, :],
                    lhsT=q_emb8[:, fhi0:fhi0 + 2, c * 128:(c + 1) * 128],
                    rhs=dpk_sb,
                    start=True,
                    stop=True,
                    perf_mode=DR,
                    skip_group_check=True,
                )
            G2 = band_pool.tile([128, NT, 16], BF16, tag="g2")
            nc.scalar.activation(
                out=G2, in_=psb[:, 8:16, :], func=mybir.ActivationFunctionType.Exp
            )
            # BT[q, c, u] = B9[q-u, 1+c, u] (+ chunk borrow via ghost slot 0)
            for u in range(16):
                nc.tensor.matmul(
                    psb[:, 0:8, u],
                    lhsT=bigm_sb[:, 128 - u:256 - u],
                    rhs=B9[:, 1:9, u],
                    start=True,
                    stop=(u == 0),
                    skip_group_check=True,
                )
                if u > 0:
                    nc.tensor.matmul(
                        psb[:, 0:8, u],
                        lhsT=bigm_sb[:, 256 - u:384 - u],
                        rhs=B9[:, 0:8, u],
                        start=False,
                        stop=True,
                        skip_group_check=True,
                    )
            WB_sb = band_pool.tile([128, NT, 16], F32, tag="wb")
            nc.vector.tensor_tensor(
                out=WB_sb, in0=psb[:, 0:8, :], in1=G2, op=mybir.AluOpType.mult
            )
            CT9 = band_pool.tile([128, 9, 16], BF16, tag="ct9")
            nc.scalar.memzero(CT9[:, 8, :])
            nc.vector.tensor_tensor(
                out=CT9[:, 0:8, :], in0=WB_sb, in1=psb[:, 0:8, :],
                op=mybir.AluOpType.subtract,
            )
            # corr_s[p, c, u] = CT9[p+u, c or c+1, u]
            for u in range(16):
                nc.tensor.matmul(
                    psb[:, 0:8, u],
                    lhsT=bigm_sb[:, 128 + u:256 + u],
                    rhs=CT9[:, 0:8, u],
                    start=True,
                    stop=(u == 0),
                    skip_group_check=True,
                )
                if u > 0:
                    nc.tensor.matmul(
                        psb[:, 0:8, u],
                        lhsT=bigm_sb[:, u:128 + u],
                        rhs=CT9[:, 1:9, u],
                        start=False,
                        stop=True,
                        skip_group_check=True,
                    )
            CS_sb = band_pool.tile([128, NT, 16], BF16, tag="cs")
            nc.scalar.copy(out=CS_sb, in_=psb[:, 0:8, :])
            # scatter-accumulate the correction runs into the staged window
            nc.gpsimd.dma_start(
                out=dram_ap(wstg, 0, [[RW + 1, 128], [EXT_W, NT], [1, 16]]),
                in_=CS_sb,
                accum_op=mybir.AluOpType.add,
            )
            # re-read the corrected windows into a dedicated bw tile
            bw = bw_pool.tile([128, NT, EXT_W], BF16, tag="bw")
            nc.sync.dma_start(
                out=bw, in_=dram_ap(wstg, 0, [[RW, 128], [1, RW]])
            )
            # wband_all[(c,u), t'] via PE transpose of WB_sb
            nc.tensor.transpose(psb[:, 16:24, :], WB_sb, idf_sb)
            WA_sb = band_pool.tile([128, 128], BF16, tag="wa")
            nc.scalar.copy(out=WA_sb, in_=psb[:, 16:24, :])
            nc.scalar.dma_start(
                out=dram_ap(wb_stage[h], 0, [[128, 128], [1, 128]]), in_=WA_sb
            )
            wband_sb = band_pool.tile([M, NT, 128], BF16, tag="wband")
            nc.sync.dma_start(
                out=wband_sb,
                in_=dram_ap(wb_stage[h], 0, [[128, M], [2048, NT], [1, 128]]),
            )
            st[3], st[4] = bw, wband_sb

        def emit_band_av(h):
            """Stage C (PE): band AV from the corrected windows + rel-v."""
            st = state[h]
            pof, bw, wband_sb = st[2], st[3], st[4]
            for th in range(2):
                for (c, lo, hi) in BANDAV[th]:
                    s0 = c * 128
                    nc.tensor.matmul(
                        pof[:, lo:hi],
                        lhsT=v_pad[:, c, h, 0:65],
                        rhs=bw[:, c, lo - s0:hi - s0],
                        start=False,
                        stop=False,
                        skip_group_check=True,
                    )
                nc.tensor.matmul(
                    pof[0:64, th * 512:(th + 1) * 512],
                    lhsT=dpvt_sb,
                    rhs=wband_sb[:, th * 4:(th + 1) * 4, :],
                    start=False,
                    stop=(th == 1),
                    skip_group_check=True,
                )

        def emit_recip(h):
            """Denominator reciprocal + broadcast.  Steady-state heads use
            a DRAM round trip; the tail heads (scores psum free by then)
            broadcast via an fp32r ones-matmul so nothing rides the tail."""
            st = state[h]
            pof = st[2]
            if h >= H - 2:
                nrm = nrm_pool.tile([1, T], BF16, tag="rdenb16")
                with nc.allow_low_precision("softmax 1/den at bf16 is plenty"):
                    nc.vector.reciprocal(out=nrm, in_=pof[64:65, :])
                nrmb = nrm_pool.tile([64, T], F32, tag="rdenb")
                for th in range(2):
                    psn = ps_main.tile([128, 512], F32, tag="msc")
                    nc.tensor.matmul(
                        psn[0:64, :],
                        lhsT=ones_row[:, 0:64],
                        rhs=nrm[:, th * 512:(th + 1) * 512],
                        start=True,
                        stop=True,
                        skip_group_check=True,
                    )
                    nc.scalar.copy(
                        out=nrmb[:, th * 512:(th + 1) * 512], in_=psn[0:64, :]
                    )
                st.append(nrmb)
            else:
                nrm = nrm_pool.tile([1, T], F32, tag="rden")
                nc.vector.reciprocal(out=nrm, in_=pof[64:65, :])
                nc.gpsimd.dma_start(
                    out=dram_ap(rden_stage[h], 0, [[T, 1], [1, T]]), in_=nrm,
                )
                nrmb = nrm_pool.tile([64, T], F32, tag="rdenb")
                nc.gpsimd.dma_start(
                    out=nrmb, in_=dram_ap(rden_stage[h], 0, [[0, 64], [1, T]]),
                )
                st.append(nrmb)

        def emit_merged(h):
            """Normalize into mergedT once the broadcast lands."""
            wt_t, B9, pof, bw, wband_sb, nrmb = state.pop(h)
            hb = 64 * (h % 2)
            co = h // 2
            nc.vector.tensor_tensor(
                out=mergedT[co][hb:hb + 64, :],
                in0=pof[0:64, :],
                in1=nrmb,
                op=mybir.AluOpType.mult,
            )

        NP = len(SCORE_PIECES)
        B1, B2 = 0, 10
        emit_qk_co(Wq_sb, qT_sb, bq_sb, q_embT8, q_emb8, q_stage, 0, False)
        emit_qk_co(Wk_sb, kT_sb, bk_sb, k_embT8, k_emb8, k_stage, 0, True)
        emit_scores_open(0)
        emit_scores_part(0, 0, B2)
        for co in range(1, 4):
            emit_qk_co(Wq_sb, qT_sb, bq_sb, q_embT8, q_emb8, q_stage, co, False)
            emit_qk_co(Wk_sb, kT_sb, bk_sb, k_embT8, k_emb8, k_stage, co, True)
        emit_scores_part(0, B2, NP)
        emit_scores_close(0)
        for ti in range(NT):
            emit_v_ti(ti)
        nc.vector.memset(v_pad[:, :, :, 64:65], 1.0)
        for i in range(1, H + 2):
            if i < H:
                emit_scores_open(i)
                emit_scores_part(i, 0, B1)
            if 1 <= i < H + 1:
                emit_far_av(i - 1)
            if i < H:
                emit_scores_part(i, B1, B2)
            if i >= 2 and i - 2 < H:
                emit_band_av(i - 2)
                emit_recip(i - 2)
            if i < H:
                emit_scores_part(i, B2, NP)
                emit_scores_close(i)
            if 1 <= i < H + 1:
                emit_band(i - 1)
            if i >= 2 and i - 2 < H:
                emit_merged(i - 2)
            if i == 7:
                nc.gpsimd.dma_start(
                    out=qn_sb[:, 0:4, :],
                    in_=dram_ap(p_qn, 0, [[U, 128], [128 * U, 4], [1, U]]),
                )
                nc.gpsimd.dma_start(
                    out=qn_sb[:, 4:8, :],
                    in_=dram_ap(p_qn, 4 * 128 * U, [[U, 128], [128 * U, 4], [1, U]]),
                )

        # ---- phase 3: output projection + residual + layernorm ----------
        y_lo = y_pool.tile([128, 4, U], BF16, tag="ylo")
        y_hi = y_pool.tile([128, 4, U], BF16, tag="yhi")

        # early partials: heads 0-5 columns (ci 0-2) for the first tiles
        # overlap the last head's band-correction chain
        NEARLY = 3
        early_ps = []
        for ti in range(NEARLY):
            ps = ps_main.tile([128, 512], F32, tag="msc")
            early_ps.append(ps)
            for ci in range(3):
                nc.tensor.matmul(
                    ps,
                    lhsT=mergedT[ci][:, ti * 128:(ti + 1) * 128],
                    rhs=Wo_sb[:, ci, :],
                    start=(ci == 0),
                    stop=False,
                    skip_group_check=True,
                )
        for ti in range(NT):
            if ti < NEARLY:
                ps = early_ps[ti]
                nc.tensor.matmul(
                    ps,
                    lhsT=mergedT[3][:, ti * 128:(ti + 1) * 128],
                    rhs=Wo_sb[:, 3, :],
                    start=False,
                    stop=False,
                    skip_group_check=True,
                )
            else:
                ps = ps_main.tile([128, 512], F32, tag="msc")
                for ci in range(4):
                    nc.tensor.matmul(
                        ps,
                        lhsT=mergedT[ci][:, ti * 128:(ti + 1) * 128],
                        rhs=Wo_sb[:, ci, :],
                        start=(ci == 0),
                        stop=False,
                        skip_group_check=True,
                    )
            nc.tensor.matmul(ps, lhsT=ones_row, rhs=bo_sb, start=False, stop=True,
                             skip_group_check=True)
            x = ln_pool.tile([128, U], BF16, tag="x")
            nc.scalar.activation(
                out=x, in_=ps, func=mybir.ActivationFunctionType.Relu
            )
            adder = nc.vector
            adder.tensor_tensor(
                out=x, in0=x, in1=qn_sb[:, ti, :], op=mybir.AluOpType.add
            )
            stats = ln_pool.tile([128, 6], F32, tag="st")
            nc.vector.bn_stats(out=stats, in_=x)
            mv = ln_pool.tile([128, 2], F32, tag="mv")
            nc.vector.bn_aggr(out=mv, in_=stats)
            rstd = ln_pool.tile([128, 1], F32, tag="rs")
            nc.scalar.activation(
                out=rstd,
                in_=mv[:, 1:2],
                func=mybir.ActivationFunctionType.Sqrt,
                bias=eps_sb,
            )
            nc.vector.reciprocal(out=rstd, in_=rstd)
            y = y_lo[:, ti, :] if ti < 4 else y_hi[:, ti - 4, :]
            tse = nc.vector
            tse.tensor_scalar(
                out=y,
                in0=x,
                scalar1=mv[:, 0:1],
                scalar2=rstd,
                op0=mybir.AluOpType.subtract,
                op1=mybir.AluOpType.mult,
            )
            if apply_gamma_beta:
                nc.vector.tensor_tensor(
                    out=y, in0=y, in1=gam_bc, op=mybir.AluOpType.mult
                )
                nc.vector.tensor_tensor(
                    out=y, in0=y, in1=bet_bc, op=mybir.AluOpType.add
                )
            if ti % 2 == 1:
                half, qi = divmod(ti - 1, 4)
                yq = nc.scalar if ti % 4 == 1 else nc.sync
                yq.dma_start(
                    out=dram_ap(p_out, (ti - 1) * 128 * U,
                                [[U, 128], [128 * U, 2], [1, U]]),
                    in_=(y_lo[:, qi:qi + 2, :] if half == 0 else y_hi[:, qi:qi + 2, :]),
                )

        cstack.close()

    split_excess_waits(nc)
    return nc


_NC_CACHE = {}


def _get_nc(apply_gamma_beta, has_bv=False):
    key = (bool(apply_gamma_beta), bool(has_bv))
    if key not in _NC_CACHE:
        _NC_CACHE[key] = build_nc(*key)
    return _NC_CACHE[key]


def _pack_dr(W):
    """[512, N] -> [128, 2j, 2i, N] -> [128, 4N] with k = 256j + 128i + p."""
    N = W.shape[1]
    return np.ascontiguousarray(
        W.reshape(2, 2, 128, N).transpose(2, 0, 1, 3).reshape(128, 4 * N)
    )


def kernel(q, k, v, Wq, bq, Wk, bk, Wv, bv, Wo, bo, gamma, beta, pe_k, pe_v):
    q = np.asarray(q, np.float32)
    k = np.asarray(k, np.float32)
    v = np.asarray(v, np.float32)
    Wq = np.asarray(Wq, np.float32)
    Wk = np.asarray(Wk, np.float32)
    Wv = np.asarray(Wv, np.float32)
    Wo = np.asarray(Wo, np.float32)
    bq = np.asarray(bq, np.float32)
    bk = np.asarray(bk, np.float32)
    bv = np.asarray(bv, np.float32)
    bo = np.asarray(bo, np.float32)
    gamma = np.asarray(gamma, np.float32)
    beta = np.asarray(beta, np.float32)
    pe_k = np.asarray(pe_k, np.float32)
    pe_v = np.asarray(pe_v, np.float32)

    bf = ml_dtypes.bfloat16
    f8 = ml_dtypes.float8_e4m3

    trivial = bool(np.all(gamma == 1.0) and np.all(beta == 0.0))
    has_bv = bool(np.any(bv != 0.0))
    nc = _get_nc(not trivial, has_bv)

    # dpk_rev[:, u] = (pe_k[16-u] - pe_k[0]) / 8,  u in [0, 16)
    dpe_k = (pe_k[1:17] - pe_k[0])[::-1].T * SM_SCALE   # [64, 16]
    dpk8 = np.ascontiguousarray(
        dpe_k.reshape(2, 32, 16).transpose(1, 0, 2).reshape(32, 32)
    )
    # dpv_rev rows u = pe_v[16-u] - pe_v[0]
    dpe_v = (pe_v[1:17] - pe_v[0])[::-1]             # [16, 64]
    bigm = np.zeros((128, 384), np.float32)
    bigm[np.arange(128), np.arange(128) + 128] = 1.0

    shared = {
        "Wq8": _pack_dr(Wq).astype(f8),
        "Wk8": _pack_dr(Wk).astype(f8),
        "Wv8": _pack_dr(Wv).astype(f8),
        "Wo": Wo.astype(bf),
        "bq_c": bq.reshape(U, 1),
        "bk_c": bk.reshape(U, 1),
        "bv_row": bv.astype(bf).reshape(1, U),
        "bo_row": (bo + np.tile(pe_v[0], H) @ Wo).astype(bf).reshape(1, U),
        "dpk8": dpk8.astype(f8),
        "dpvr": np.ascontiguousarray(dpe_v).astype(bf),
        "bigm": bigm.astype(bf),
        "identf": np.eye(128, dtype=np.float32),
        "cmask": np.triu(np.ones((128, 128), np.float32)).astype(bf),
        "gamma_r": gamma.reshape(1, U).astype(np.float32),
        "beta_r": beta.reshape(1, U).astype(np.float32),
    }

    in_maps = []
    for b_i in range(B):
        m = dict(shared)
        m["qT8"] = _pack_dr(np.ascontiguousarray(q[b_i].T)).astype(f8)
        m["kT8"] = _pack_dr(np.ascontiguousarray(k[b_i].T)).astype(f8)
        m["vT8"] = _pack_dr(np.ascontiguousarray(v[b_i].T)).astype(f8)
        m["qn"] = q[b_i].astype(bf)
        in_maps.append(m)

    res = run_bass_kernel_spmd(nc, in_maps, core_ids=list(range(B)))
    global LAST_RESULT
    LAST_RESULT = res
    out = np.stack([np.asarray(res.results[b_i]["out"], np.float32) for b_i in range(B)], axis=0)
    return out


LAST_RESULT = None
